# revision 23
# baseline (speedup 1.0000x reference)
"""Trainium2 Bass kernel for nn_Attention_29738353557815.

8-way tensor-parallel over heads:
  - core c owns q-heads {2c, 2c+1} and kv-head c//2 (k/v proj duplicated per core pair)
  - fp16 datapath: hT/wqg/wkv/wo/tables host-cast to fp16 (halves DMA, full PE rate);
    PSUM accumulation stays fp32
  - projections run weights-stationary off host-pretransposed hidden^T in [128, 1024]
    column-major tiles so the first proj chunk is fed after ~14us of DMA
  - h0 attention is interleaved chunk-by-chunk with the projections (chunk j's
    attention only needs k/v for s <= 512(j+1), available after proj chunk j), so
    the h0 AllToAll launches right at proj end and the h1 AllToAll right after
    h1 attention -- the two collectives pipeline against o-proj h0-half
  - rms-norm folded into ln/exp on ACT; rope tables (cos/sin * sqrt(scale)) are
    host-precomputed in [hd, T] fp16 layout; rotate-half via half-tile ops
  - attention in S^T layout; causal mask via gpsimd.affine_select, segment mask via
    scalar_tensor_tensor vs iota; invalid (s,t) tiles skipped at build time
  - gating: G = 1+exp(-g) stored at proj time; per-chunk epilogue is
    atg = ot * exp(-ln(rs*G)) -- 2 DVE + 2 ACT ops, no extra copies
  - all wo tiles prefetched during attention (SP queue idle there); o-proj streams
    per-ATall-tile so the tail is ATall DMA + 8 matmuls deep

All DMAs keep >=1 KiB contiguous per-partition runs (weights host-prepacked into
[128, ...] partition-major fp16 layouts).
"""
import sys

if "/opt/trn_rl_repo" not in sys.path:
    sys.path.insert(0, "/opt/trn_rl_repo")

import numpy as np

import concourse.bass as bass
from concourse import bacc
import concourse.mybir as mybir
import concourse.tile as tile
from concourse.bass_utils import run_bass_kernel_spmd
from concourse.masks import make_identity

F32 = mybir.dt.float32
F16 = mybir.dt.float16
AF = mybir.ActivationFunctionType
OP = mybir.AluOpType

B, T, D = 1, 2048, 2048
NH, NKV, HD = 16, 4, 128
EPS = 1e-6
SCALE = HD ** -0.5
NCORES = 8
P = 128
NJ = T // 512      # 4 t-chunks of 512
NT = T // P        # 16 s-tiles of 128
DT = D // P        # 16 contraction tiles
TSL = T // NCORES  # 256 output rows per core

_program_cache: dict = {}


def _tile_flags(seg_end: np.ndarray):
    """Per (s-tile i, t-chunk j): (skip, needs_causal, needs_seg)."""
    flags = []
    for i in range(NT):
        smin, smax = P * i, P * i + P - 1
        se_lo = int(seg_end[smin])
        se_hi = int(seg_end[smax])
        row = []
        for j in range(NJ):
            t0, t1 = 512 * j, 512 * j + 511
            skip = (t1 < smin) or (t0 >= se_hi)
            causal = (not skip) and (t0 < smax)
            segm = (not skip) and (t1 >= se_lo)
            row.append((skip, causal, segm))
        flags.append(row)
    return tuple(tuple(r) for r in flags)


def _build_program(key, use_collective=True):
    flags, unit_w = key
    nc = bacc.Bacc("TRN2", target_bir_lowering=False, debug=False,
                   num_devices=NCORES)

    hT_d = nc.dram_tensor("hT", [P, DT, T], F16, kind="ExternalInput")
    wqg_d = nc.dram_tensor("wqg", [P, DT, 512], F16, kind="ExternalInput")
    wkv_d = nc.dram_tensor("wkv", [P, DT, 256], F16, kind="ExternalInput")
    wo_d = nc.dram_tensor("wo", [P, NT, 2048], F16, kind="ExternalInput")
    tblq_d = nc.dram_tensor("tblq", [2, P, T], F16, kind="ExternalInput")
    if not unit_w:
        wqk_d = nc.dram_tensor("wqk", [P, 2], F16, kind="ExternalInput")
    iota_d = nc.dram_tensor("iota", [P, 512], F16, kind="ExternalInput")
    segrel_d = nc.dram_tensor("segrel", [P, NT, NJ], F16, kind="ExternalInput")
    caurel_d = nc.dram_tensor("caurel", [P, NT, NJ], F16, kind="ExternalInput")
    out_d = nc.dram_tensor("out", [TSL, D], F16, kind="ExternalOutput")

    with tile.TileContext(nc) as tc:
        with (
            tc.tile_pool(name="consts", bufs=1) as consts,
            tc.tile_pool(name="perm", bufs=1) as perm,
            tc.tile_pool(name="hw", bufs=32) as hw,
            tc.tile_pool(name="wop", bufs=16) as wop,
            tc.tile_pool(name="tmp", bufs=5) as tmp,
            tc.tile_pool(name="ptp", bufs=5) as ptp,
            tc.tile_pool(name="ps", bufs=1, space="PSUM") as psp,
            tc.tile_pool(name="dram", bufs=1, space="DRAM") as dram,
        ):
            # ---- constants; DMA emission of the big ones is interleaved with
            # the first hT tiles so the first matmul starts early
            wqg_sb = [consts.tile([P, 4, 512], F16, tag="wqg", bufs=4,
                                  name=f"wqg{g}") for g in range(4)]
            wkv_sb = [consts.tile([P, 8, 256], F16, tag="wkv", bufs=2,
                                  name=f"wkv{g}") for g in range(2)]

            def wq_ap(dt, col0):
                return wqg_sb[dt // 4][:, dt % 4, col0:col0 + 128]

            def wkv_ap(dt, col0):
                return wkv_sb[dt // 8][:, dt % 8, col0:col0 + 128]

            tb = {}
            tb_srcs = []
            for nm, idx in (("cq", 0), ("sq", 1)):
                t_ = consts.tile([P, T], F16, tag=f"tb_{nm}", name=f"tb_{nm}")
                tb_srcs.append((t_, tblq_d, idx))
                tb[nm] = t_
            if not unit_w:
                wqk_sb = consts.tile([P, 2], F16)
                nc.sync.dma_start(wqk_sb[:], wqk_d[:])
            iota_sb = consts.tile([P, 512], F16)
            segrel_sb = consts.tile([P, NT, NJ], F16)
            caurel_sb = consts.tile([P, NT, NJ], F16)
            ones_f32 = consts.tile([P, P], F32)
            nc.vector.memset(ones_f32[:], 1.0)
            ones_sb = consts.tile([P, P], F16)
            nc.vector.tensor_copy(ones_sb[:], ones_f32[:])
            ident_f32 = consts.tile([P, P], F32)
            make_identity(nc, ident_f32[:])
            ident_sb = consts.tile([P, P], F16)
            nc.vector.tensor_copy(ident_sb[:], ident_f32[:])
            eps_sb = consts.tile([P, 1], F32)
            nc.vector.memset(eps_sb[:], EPS)
            # prime the ACT function-table load at t~0 (the dedupe pass keeps
            # only the first load; anchoring it here takes its ~2.7us off the
            # first rms-norm chain)
            actwarm = consts.tile([P, 1], F32)
            nc.scalar.activation(actwarm[:], eps_sb[:], AF.Copy)

            # ---- persistent activations ----
            qTr = [perm.tile([P, T], F16, tag=f"qTr{h}", name=f"qTr{h}")
                   for h in range(2)]
            kTr = perm.tile([P, T], F16, tag="kTr")
            gG = [perm.tile([P, T], F16, tag=f"gG{h}", name=f"gG{h}")
                  for h in range(2)]
            v_sb = perm.tile([P, NT, P], F16, tag="v_sb")

            # split A2A by head: h0's collective runs during h1 attention
            a2a_in = [dram.tile([NCORES * P, TSL], F16, name=f"a2a_in{h}")
                      for h in range(2)]
            a2a_in8 = [a.rearrange("(s r) t -> s r t", r=P) for a in a2a_in]
            a2a_out = [dram.tile([NCORES * P, TSL], F16, name=f"a2a_out{h}")
                       for h in range(2)]

            def emit_attention(h, j):
                tsl = slice(512 * j, 512 * j + 512)
                valid = [i for i in range(NT) if not flags[i][j][0]]
                last = len(valid) - 1
                ot_ps = psp.tile([P, 512], F32, tag="acc", bufs=4,
                                 name=f"ot_{h}_{j}")
                rs_ps = psp.tile([P, 512], F32, tag="acc", bufs=4,
                                 name=f"rs_{h}_{j}")
                for idx, i in enumerate(valid):
                    _, needs_c, needs_s = flags[i][j]
                    st_ps = psp.tile([P, 512], F32, tag="mm", bufs=3,
                                     name=f"st_{h}_{j}_{i}")
                    nc.tensor.matmul(st_ps[:], kTr[:, P * i:P * i + P],
                                     qTr[h][:, tsl], start=True, stop=True)
                    pt = ptp.tile([P, 512], F16, tag="pt", name=f"pt_{h}_{j}_{i}")
                    nc.scalar.activation(pt[:], st_ps[:], AF.Exp)
                    if needs_c:
                        if h == 0 or j < 2:
                            # Pool is free until the h0 collective dispatches
                            nc.gpsimd.affine_select(
                                out=pt[:], in_=pt[:], pattern=[[1, 512]],
                                compare_op=OP.is_ge, fill=0.0,
                                base=512 * j - P * i, channel_multiplier=-1)
                        else:
                            # the h0 collective blocks the Pool queue for its
                            # whole duration -- late h1 causal masking on DVE
                            nc.vector.scalar_tensor_tensor(
                                out=pt[:], in0=iota_sb[:],
                                scalar=caurel_sb[:, i, j:j + 1], in1=pt[:],
                                op0=OP.is_ge, op1=OP.mult)
                    if needs_s:
                        nc.vector.scalar_tensor_tensor(
                            out=pt[:], in0=iota_sb[:],
                            scalar=segrel_sb[:, i, j:j + 1], in1=pt[:],
                            op0=OP.is_lt, op1=OP.mult)
                    nc.tensor.matmul(ot_ps[:], v_sb[:, i, :], pt[:],
                                     start=(idx == 0), stop=(idx == last))
                    nc.tensor.matmul(rs_ps[:], ones_sb[:], pt[:],
                                     start=(idx == 0), stop=(idx == last))

                # atg = ot * sig(g)/rowsum = ot * exp(-ln(rowsum * (1+e^-g)))
                den = tmp.tile([P, 512], F32, tag="tmp", name=f"den_{h}_{j}")
                nc.vector.tensor_tensor(den[:], rs_ps[:], gG[h][:, tsl], OP.mult)
                nc.scalar.activation(den[:], den[:], AF.Ln)
                nc.scalar.activation(den[:], den[:], AF.Exp, scale=-1.0)
                atg = tmp.tile([P, 512], F16, tag="tmp2", bufs=2,
                               name=f"atg_{h}_{j}")
                nc.vector.tensor_tensor(atg[:], ot_ps[:], den[:], OP.mult)
                # stage into a2a_in[h]: chunk j covers shards 2j and 2j+1.
                # high_priority keeps these ahead of the sem-waiting ATall
                # loads in the scheduler's SP ordering.
                with tc.high_priority():
                    for half in range(2):
                        nc.sync.dma_start(
                            a2a_in8[h][2 * j + half, :, :],
                            atg[:, 256 * half:256 * half + 256])

            # ================= hT + weights DMA =================
            # [128, 1024] fp16 tiles, column-half-major: chunk 0's operands all
            # land after ~16 tiles (~14us), so proj starts ~immediately
            hTt = [[None] * DT for _ in range(2)]
            for h2 in range(2):
                for dt in range(DT):
                    if h2 == 0 and dt % 4 == 0:
                        g = dt // 4
                        nc.sync.dma_start(wqg_sb[g][:],
                                          wqg_d[:, 4 * g:4 * g + 4, :])
                    t_ = hw.tile([P, 1024], F16, tag="hw",
                                 name=f"hT_{h2}_{dt}")
                    nc.sync.dma_start(
                        t_[:], hT_d[:, dt, 1024 * h2:1024 * h2 + 1024])
                    hTt[h2][dt] = t_
                    if h2 == 0 and dt == 10:
                        for g in range(2):
                            nc.sync.dma_start(wkv_sb[g][:],
                                              wkv_d[:, 8 * g:8 * g + 8, :])
                    if h2 == 0 and dt == 13:
                        for t2_, dsrc, idx in tb_srcs:
                            nc.sync.dma_start(t2_[:], dsrc[idx])
                        nc.sync.dma_start(iota_sb[:], iota_d[:])
                        nc.sync.dma_start(segrel_sb[:], segrel_d[:])
                        nc.sync.dma_start(caurel_sb[:], caurel_d[:])

            # ================= proj chunk j (+ inline h0 attention) ==========
            def _proj_w_ap(c):
                if c < 2:
                    return lambda dt, c=c: wq_ap(dt, 128 * c)
                elif c == 2:
                    return lambda dt: wkv_ap(dt, 0)
                elif c == 3:
                    return lambda dt: wkv_ap(dt, 128)
                return lambda dt, c=c: wq_ap(dt, 256 + 128 * (c - 4))

            def emit_proj(j):
                h2, jj = j // 2, j % 2
                tsl = slice(512 * j, 512 * j + 512)
                hsl = slice(512 * jj, 512 * jj + 512)
                hTj = hTt[h2]

                banks = {}
                if j == 0:
                    # dt-major across 6 live banks: each hT tile is consumed
                    # once, right behind its DMA -- no mid-chunk PE stalls
                    # while the first 16 tiles stream in
                    for c, (tg, bf) in zip(
                            (0, 1, 4, 5, 2, 3),
                            [("mm", 3)] * 3 + [("acc", 4)] * 3):
                        banks[c] = psp.tile([P, 512], F32, tag=tg, bufs=bf,
                                            name=f"proj_{j}_{c}")
                    for dt in range(DT):
                        for c in (0, 1, 4, 5, 2, 3):
                            nc.tensor.matmul(banks[c][:], _proj_w_ap(c)(dt),
                                             hTj[dt][:, hsl],
                                             start=(dt == 0),
                                             stop=(dt == DT - 1))

                # order: q0 q1 g0 g1 k v (k/v last -> slack for wkv DMA)
                for c in (0, 1, 4, 5, 2, 3):
                    w_ap = _proj_w_ap(c)
                    if j == 0:
                        mm_ps = banks[c]
                    else:
                        ptag, pbufs = (("mm", 3) if c in (0, 1, 4, 5)
                                       else ("acc", 4))
                        mm_ps = psp.tile([P, 512], F32, tag=ptag, bufs=pbufs,
                                         name=f"proj_{j}_{c}")
                        for dt in range(DT):
                            nc.tensor.matmul(mm_ps[:], w_ap(dt),
                                             hTj[dt][:, hsl],
                                             start=(dt == 0),
                                             stop=(dt == DT - 1))

                    if c in (0, 1, 2):  # q0/q1/k: rms-norm + rope
                        dest = qTr[c][:, tsl] if c < 2 else kTr[:, tsl]
                        qpre = tmp.tile([P, 512], F16, tag="tmp")
                        nc.vector.tensor_copy(qpre[:], mm_ps[:])
                        q2 = ptp.tile([P, 512], F16, tag="pt")
                        nc.scalar.activation(q2[:], mm_ps[:], AF.Square)
                        if not unit_w:
                            # norm weight applied after the rms statistic,
                            # before rope (rope commutes with rsqrt only)
                            qw = tmp.tile([P, 512], F16, tag="tmp")
                            nc.vector.tensor_scalar_mul(
                                qw[:], qpre[:],
                                wqk_sb[:, (0 if c < 2 else 1):
                                       (1 if c < 2 else 2)])
                            qpre = qw
                        ssq_ps = psp.tile([P, 512], F32, tag="aux", bufs=1)
                        nc.tensor.matmul(ssq_ps[:], ones_sb[:], q2[:],
                                         start=True, stop=True)
                        rsv = tmp.tile([P, 512], F16, tag="tmp")
                        nc.scalar.activation(rsv[:], ssq_ps[:], AF.Ln,
                                             scale=1.0 / HD, bias=eps_sb[:, 0:1])
                        nc.scalar.activation(rsv[:], rsv[:], AF.Exp, scale=-0.5)
                        tcos = tmp.tile([P, 512], F16, tag="tmp")
                        nc.vector.tensor_tensor(tcos[:], qpre[:], tb["cq"][:, tsl],
                                                OP.mult)
                        t2 = tmp.tile([P, 512], F16, tag="tmp")
                        # sin table halves are pre-swapped host-side so both
                        # inputs share a base partition; only out is shifted
                        nc.vector.tensor_tensor(t2[0:64, :], qpre[64:128, :],
                                                tb["sq"][64:128, tsl], OP.mult)
                        nc.vector.tensor_tensor(t2[64:128, :], qpre[0:64, :],
                                                tb["sq"][0:64, tsl], OP.mult)
                        nc.vector.tensor_tensor(t2[:], tcos[:], t2[:], OP.add)
                        nc.vector.tensor_tensor(dest, t2[:], rsv[:], OP.mult)
                    elif c in (4, 5):  # gate: store G = 1 + exp(-g)
                        eg = tmp.tile([P, 512], F16, tag="tmp")
                        nc.scalar.activation(eg[:], mm_ps[:], AF.Exp,
                                             scale=-1.0)
                        nc.scalar.activation(gG[c - 4][:, tsl], eg[:],
                                             AF.Copy, bias=1.0)
                    else:  # v: transpose [hd, t] -> [t, hd] tiles
                        vtmp = tmp.tile([P, 512], F16, tag="tmp")
                        nc.vector.tensor_copy(vtmp[:], mm_ps[:])
                        for kk in range(4):
                            tt = 4 * j + kk
                            trp = psp.tile([P, P], F16, tag="aux", bufs=1)
                            nc.tensor.transpose(
                                trp[:], vtmp[:, 128 * kk:128 * kk + 128],
                                ident_sb[:])
                            nc.vector.tensor_copy(v_sb[:, tt, :], trp[:])

            # wo prefetch helpers: h0 tiles get the dedicated pool; h1 tiles
            # reuse freed hT slots (2 KiB each, so two per ht block). Emitted
            # inside the proj loop so the DMA engines are done with wo well
            # before the a2a staging / ATall traffic needs them.
            wo_slices = []

            def emit_wo_prefetch():
                for ht in range(8):
                    w_ = wop.tile([P, 2048], F16, tag="wop", bufs=8,
                                  name=f"wo_{ht}")
                    nc.sync.dma_start(w_[:], wo_d[:, ht, :])
                    wo_slices.append([w_[:, 512 * Dc:512 * Dc + 512]
                                      for Dc in range(NJ)])
                for ht in range(8, NT):
                    sl = []
                    for wh in range(2):
                        w_ = hw.tile([P, 1024], F16, tag="hw",
                                     name=f"wo_{ht}_{wh}")
                        nc.sync.dma_start(
                            w_[:], wo_d[:, ht, 1024 * wh:1024 * wh + 1024])
                        sl += [w_[:, 0:512], w_[:, 512:1024]]
                    wo_slices.append(sl)

            for j in range(NJ):
                emit_proj(j)
                if j == 2:
                    emit_wo_prefetch()
                emit_attention(0, j)

            # h1 chunks 0/1 before the h0 collective: their Pool affines can
            # still run ahead of the collective's Pool-queue hold
            emit_attention(1, 0)
            emit_attention(1, 1)

            if use_collective:
                nc.gpsimd.collective_compute(
                    "AllToAll", OP.bypass,
                    replica_groups=[list(range(NCORES))],
                    ins=[a2a_in[0][:].opt()], outs=[a2a_out[0][:].opt()])
            else:
                nc.sync.dma_start(a2a_out[0][:], a2a_in[0][:])

            emit_attention(1, 2)
            emit_attention(1, 3)
            if use_collective:
                nc.gpsimd.collective_compute(
                    "AllToAll", OP.bypass,
                    replica_groups=[list(range(NCORES))],
                    ins=[a2a_in[1][:].opt()], outs=[a2a_out[1][:].opt()])
            else:
                nc.sync.dma_start(a2a_out[1][:], a2a_in[1][:])

            # ================= o-proj =================
            # ATall loads emitted after BOTH collectives so the h1 staging DMAs
            # are not stuck behind the sem-wait on a2a_out0 in the SP queue
            # ATall loads go on the Pool queue (idle once the affine-selects
            # are done; the collectives free their SEQ before the CC wait) --
            # on SP or Act their sem-wait head-of-line blocks live traffic.
            # o-proj is emitted in two ht-halves so the h0 half's matmuls sit
            # ahead of the h1 ATall waits in the PE queue.
            ops_tags = ["mm", "mm", "mm", "aux", "acc", "acc", "acc", "acc"]
            ops_bufs = {"mm": 3, "aux": 1, "acc": 4}
            ops = []
            for m in range(2):
                for Dc in range(NJ):
                    tg = ops_tags[m * NJ + Dc]
                    ops.append(psp.tile([P, 512], F32, tag=tg,
                                        bufs=ops_bufs[tg], name=f"ops{m}_{Dc}"))
            ATall = []
            for h in range(2):
                for i in range(8):
                    at_t = perm.tile([P, TSL], F16, tag="ATall", bufs=16,
                                     name=f"ATall{8 * h + i}")
                    nc.sync.dma_start(at_t[:], a2a_out[h][P * i:P * i + P, :])
                    ATall.append(at_t)
                for ht in range(8 * h, 8 * h + 8):
                    at_t = ATall[ht]
                    for Dc in range(NJ):
                        for m in range(2):
                            nc.tensor.matmul(
                                ops[m * NJ + Dc][:],
                                at_t[:, 128 * m:128 * m + 128],
                                wo_slices[ht][Dc],
                                start=(ht == 0), stop=(ht == NT - 1))
            # assemble [128, 1024] halves so the final writes have 4 KiB dram
            # runs instead of floor-bound 2 KiB ones; copies alternate DVE/Act
            # so the drain is two-wide
            for m in range(2):
                for Dh in range(2):
                    o_sb = hw.tile([P, 1024], F16, tag="ot", bufs=4,
                                   name=f"o_{m}_{Dh}")
                    nc.vector.tensor_copy(o_sb[:, 0:512],
                                          ops[m * NJ + 2 * Dh][:])
                    nc.scalar.activation(o_sb[:, 512:1024],
                                         ops[m * NJ + 2 * Dh + 1][:], AF.Copy)
                    nc.sync.dma_start(
                        out_d[128 * m:128 * m + 128,
                              1024 * Dh:1024 * Dh + 1024], o_sb[:])

    nc.compile()
    _dedupe_act_table_loads(nc)
    return nc


def _dedupe_act_table_loads(nc):
    """Bacc assigns Exp->exp_and_others and Ln->natural_log, inserting a
    ~2.7us table load at every Exp<->Ln alternation. All activation funcs this
    kernel uses (Exp, Ln, Square, Copy) live in the natural_log_exp_and_others
    set, so keep one load of that set and drop the rest."""
    from concourse.hw_specs import get_activation_tables
    tabs = list(get_activation_tables(nc.m.arch).items())
    nl_exp = next(i for i, (nm, funcs) in enumerate(tabs)
                  if nm == "natural_log_exp_and_others")
    used = {ins.func for bb in nc.main_func.blocks for ins in bb.instructions
            if isinstance(ins, mybir.InstActivation)}
    assert used <= tabs[nl_exp][1], f"funcs {used} not all in natural_log_exp"
    first = True
    for bb in nc.main_func.blocks:
        keep = []
        for ins in bb.instructions:
            if isinstance(ins, mybir.InstLoadActFuncSet):
                assert ins.sync_info is None or (
                    not ins.sync_info.on_wait and not ins.sync_info.on_update)
                if first:
                    ins.act_func_set_id = nl_exp
                    keep.append(ins)
                    first = False
                continue
            keep.append(ins)
        bb.instructions[:] = keep


def _host_prep(hidden_BTD, cos_BTK, sin_BTK, segment_ids_BT, position_ids_BT,
               wq, wk, wv, wo, q_norm_w, k_norm_w):
    hidden = np.ascontiguousarray(np.asarray(hidden_BTD, dtype=np.float32)[0])
    cos = np.asarray(cos_BTK, dtype=np.float32)[0]
    sin = np.asarray(sin_BTK, dtype=np.float32)[0]
    seg = np.asarray(segment_ids_BT)[0]
    pos = np.asarray(position_ids_BT)[0]
    wq = np.asarray(wq, dtype=np.float32)
    wk = np.asarray(wk, dtype=np.float32)
    wv = np.asarray(wv, dtype=np.float32)
    wo = np.asarray(wo, dtype=np.float32)
    q_norm_w = np.asarray(q_norm_w, dtype=np.float32)
    k_norm_w = np.asarray(k_norm_w, dtype=np.float32)

    assert np.array_equal(pos, np.arange(T, dtype=pos.dtype)), \
        "kernel assumes position_ids == arange"
    assert np.all(np.diff(seg) >= 0), "kernel assumes sorted segment ids"

    # [P, DT, T] fp16: hT[p, dt, t] = hidden[t, 128*dt + p]
    hT = np.ascontiguousarray(
        hidden.T.reshape(DT, P, T).transpose(1, 0, 2).astype(np.float16))
    sqrtS = np.float32(np.sqrt(SCALE))
    signv = np.where(np.arange(HD) < HD // 2, -1.0, 1.0).astype(np.float32)
    shuf = (np.arange(HD) + HD // 2) % HD

    cosw = (cos.T * sqrtS).astype(np.float32)
    sinw = (sin.T * signv[:, None] * sqrtS).astype(np.float32)
    sinswap = sinw[shuf]  # halves swapped: see rotate-half ops in _build_program
    tblq = np.ascontiguousarray(np.stack([cosw, sinswap]).astype(np.float16))
    unit_w = bool(np.all(q_norm_w == 1.0) and np.all(k_norm_w == 1.0))
    wqk = np.ascontiguousarray(
        np.stack([q_norm_w, k_norm_w], axis=1).astype(np.float16))

    # prepack wo into partition-major layout; block order matches the
    # o-proj ht-step order (all h0 head-blocks, then all h1)
    permo = [2 * i + h for h in range(2) for i in range(NCORES)]
    wo_p = wo.reshape(NT, P, 2048)[permo].transpose(1, 0, 2)
    wo_p = np.ascontiguousarray(wo_p.astype(np.float16))

    seg_end = np.searchsorted(seg, seg, side="right").astype(np.int64)
    iota = np.broadcast_to(
        np.arange(512, dtype=np.float16), (P, 512)).copy()
    segrel = np.zeros((P, NT, NJ), dtype=np.float16)
    caurel = np.zeros((P, NT, NJ), dtype=np.float16)
    rows = np.arange(P, dtype=np.float64)
    for i in range(NT):
        for j in range(NJ):
            segrel[:, i, j] = (seg_end[P * i:P * i + P] - 512.0 * j).astype(
                np.float16)
            caurel[:, i, j] = (P * i + rows - 512.0 * j).astype(np.float16)

    in_maps = []
    for c in range(NCORES):
        h0, h1 = 2 * c, 2 * c + 1
        g = c // 2
        wqg = np.concatenate([
            wq[:, h0 * 256: h0 * 256 + 128],
            wq[:, h1 * 256: h1 * 256 + 128],
            wq[:, h0 * 256 + 128: h0 * 256 + 256],
            wq[:, h1 * 256 + 128: h1 * 256 + 256],
        ], axis=1)
        wqg_p = np.ascontiguousarray(
            wqg.reshape(DT, P, 512).transpose(1, 0, 2).astype(np.float16))
        wkv = np.concatenate([
            wk[:, g * 128:(g + 1) * 128], wv[:, g * 128:(g + 1) * 128]], axis=1)
        wkv_p = np.ascontiguousarray(
            wkv.reshape(DT, P, 256).transpose(1, 0, 2).astype(np.float16))
        m = {
            "hT": hT, "wqg": wqg_p, "wkv": wkv_p, "wo": wo_p,
            "tblq": tblq, "iota": iota, "segrel": segrel, "caurel": caurel,
        }
        if not unit_w:
            m["wqk"] = wqk
        in_maps.append(m)
    return in_maps, seg_end, unit_w


def kernel(**inputs) -> np.ndarray:
    in_maps, seg_end, unit_w = _host_prep(**inputs)
    key = (_tile_flags(seg_end), unit_w)
    if key not in _program_cache:
        _program_cache[key] = _build_program(key)
    nc = _program_cache[key]
    res = run_bass_kernel_spmd(nc, in_maps, list(range(NCORES)))
    out = np.concatenate([res.results[c]["out"] for c in range(NCORES)], axis=0)
    return out[None].astype(np.float32)
